# Initial kernel scaffold
#
"""Trainium2 Bass kernel for nn_BesselBasisLayer (DimeNet radial Bessel basis).

Computes, for each edge e:
    diff = R[src[e]] - R[dst[e]]
    dist = sqrt(sum(diff^2))
    d    = dist / CUTOFF
    rbf[e, k] = envelope(d) * sin(freq_k * d)        k = 0..NUM_RADIAL-1
with envelope(x) = 1/x + a*x^5 + b*x^6 + c*x^7 (DimeNet envelope, p = 6).

Strategy: pure edge-parallel sharding across 8 NeuronCores. Each core gets a
contiguous slice of edges (src/dst indices), gathers both endpoint positions
from the full R table in HBM via indirect (descriptor-per-index) DMA, and
computes the RBF with DVE/ACT ops. The R table is passed whole to every core.
"""

import math
import sys

import numpy as np

if "/opt/trn_rl_repo" not in sys.path:
    sys.path.insert(0, "/opt/trn_rl_repo")

import concourse.bacc as bacc
import concourse.bass as bass
import concourse.mybir as mybir
from concourse.bass import IndirectOffsetOnAxis
from concourse.bass_utils import axon_active, run_bass_kernel_spmd
from concourse.tile import TileContext

F32 = mybir.dt.float32
I32 = mybir.dt.int32

N_CORES = 8
CUTOFF = 5.0
ENV_EXPONENT = 5  # DimeNet envelope_exponent; p = exponent + 1 = 6
P = 128  # SBUF partitions

MAGIC_RN = np.float32(1.5 * 2.0**23)  # float32 round-to-nearest-int trick


def _f32(x):
    return float(np.float32(x))


def _cody_waite_2pi():
    """Split 2*pi into three f32 constants with zeroed low mantissa bits so
    that k*c1 and k*c2 are exact for small integer k."""
    two_pi = 2.0 * math.pi

    def chop(x):
        b = np.float32(x).view(np.uint32)
        return (b & np.uint32(0xFFFFF000)).view(np.float32).item()

    c1 = chop(two_pi)
    c2 = chop(two_pi - c1)
    c3 = float(np.float32(two_pi - c1 - c2))
    return c1, c2, c3


def build_module(e_core, n_nodes, freqs, k_cols, num_cores=N_CORES, gather_chunks=2):
    """Build the SPMD Bass module run identically on every core.

    e_core:  edges per core; must equal 128 * k_cols * n_tiles
    n_nodes: rows of R
    freqs:   python floats, the radial frequencies (baked in at trace time)
    k_cols:  edges per partition per tile
    """
    num_radial = len(freqs)
    assert e_core % (P * k_cols) == 0, (e_core, k_cols)
    n_tiles = e_core // (P * k_cols)
    K = k_cols

    nc = bacc.Bacc(
        "TRN2",
        target_bir_lowering=False,
        debug=False,
        num_devices=num_cores,
    )

    r_t = nc.dram_tensor("R", [n_nodes, 3], F32, kind="ExternalInput")
    src_t = nc.dram_tensor("src", [e_core], I32, kind="ExternalInput")
    dst_t = nc.dram_tensor("dst", [e_core], I32, kind="ExternalInput")
    out_t = nc.dram_tensor("out", [e_core, num_radial], F32, kind="ExternalOutput")

    r_ap = r_t.ap()[:]
    # view edge arrays as [n_tiles*P, K]; tile t covers rows [t*P, (t+1)*P)
    src_v = src_t.ap()[:].rearrange("(r k) -> r k", k=K)
    dst_v = dst_t.ap()[:].rearrange("(r k) -> r k", k=K)
    # output rows for (tile t, partition p) are contiguous: K*num_radial floats
    out_v = out_t.ap()[:].rearrange("(r k) f -> r (k f)", k=K)

    # envelope coefficients (p = ENV_EXPONENT + 1)
    p_env = ENV_EXPONENT + 1
    coef_a = _f32(-(p_env + 1) * (p_env + 2) / 2.0)
    coef_b = _f32(p_env * (p_env + 2))
    coef_c = _f32(-p_env * (p_env + 1) / 2.0)

    inv_cut2 = _f32(1.0 / (CUTOFF * CUTOFF))
    c1, c2, c3 = _cody_waite_2pi()
    inv_2pi = _f32(1.0 / (2.0 * math.pi))

    TT = mybir.AluOpType
    NAN = float("nan")

    with TileContext(nc) as tc:
        with (
            tc.tile_pool(name="const", bufs=1) as cpool,
            tc.tile_pool(name="io", bufs=2) as iop,
            tc.tile_pool(name="work", bufs=2) as wp,
        ):
            nan_tile = cpool.tile([P, K], F32)
            nc.vector.memset(nan_tile[:], NAN)

            for t in range(n_tiles):
                rows = slice(t * P, (t + 1) * P)

                sidx = iop.tile([P, K], I32)
                didx = iop.tile([P, K], I32)
                nc.sync.dma_start(out=sidx[:], in_=src_v[rows])
                nc.sync.dma_start(out=didx[:], in_=dst_v[rows])

                spos = iop.tile([P, 3 * K], F32)
                dpos = iop.tile([P, 3 * K], F32)
                # chunked so descriptor generation and transfer pipeline
                cw = K // gather_chunks
                assert K % gather_chunks == 0
                for g in range(gather_chunks):
                    cs = slice(g * cw, (g + 1) * cw)
                    ps = slice(3 * g * cw, 3 * (g + 1) * cw)
                    nc.gpsimd.indirect_dma_start(
                        out=spos[:, ps],
                        out_offset=None,
                        in_=r_ap,
                        in_offset=IndirectOffsetOnAxis(ap=sidx[:, cs], axis=0),
                    )
                    nc.gpsimd.indirect_dma_start(
                        out=dpos[:, ps],
                        out_offset=None,
                        in_=r_ap,
                        in_offset=IndirectOffsetOnAxis(ap=didx[:, cs], axis=0),
                    )

                # ---- distance ----
                diff = wp.tile([P, 3 * K], F32)
                nc.vector.tensor_tensor(
                    out=diff[:], in0=spos[:], in1=dpos[:], op=TT.subtract
                )
                nc.vector.tensor_tensor(
                    out=diff[:], in0=diff[:], in1=diff[:], op=TT.mult
                )
                sq3 = diff[:].rearrange("p (k c) -> p k c", c=3)
                d2 = wp.tile([P, K], F32)
                nc.vector.tensor_tensor(
                    out=d2[:], in0=sq3[:, :, 0], in1=sq3[:, :, 1], op=TT.add
                )
                nc.vector.tensor_tensor(
                    out=d2[:], in0=d2[:], in1=sq3[:, :, 2], op=TT.add
                )
                # d = dist / CUTOFF = sqrt(d2 / CUTOFF^2)
                d = wp.tile([P, K], F32)
                nc.scalar.activation(
                    out=d[:], in_=d2[:], func=mybir.ActivationFunctionType.Sqrt,
                    scale=inv_cut2,
                )

                # ---- envelope: 1/d + a*d^5 + b*d^6 + c*d^7 (left-assoc) ----
                acc = wp.tile([P, K], F32)
                nc.vector.reciprocal(out=acc[:], in_=d[:])
                tt = wp.tile([P, K], F32)
                nc.vector.tensor_tensor(out=tt[:], in0=d[:], in1=d[:], op=TT.mult)
                pw = wp.tile([P, K], F32)
                nc.vector.tensor_tensor(out=pw[:], in0=tt[:], in1=tt[:], op=TT.mult)
                nc.vector.tensor_tensor(out=pw[:], in0=pw[:], in1=d[:], op=TT.mult)
                tmp = wp.tile([P, K], F32)
                for coef in (coef_a, coef_b, coef_c):
                    nc.vector.tensor_scalar(
                        out=tmp[:], in0=pw[:], scalar1=coef, scalar2=None, op0=TT.mult
                    )
                    nc.vector.tensor_tensor(
                        out=acc[:], in0=acc[:], in1=tmp[:], op=TT.add
                    )
                    if coef is not coef_c:
                        nc.vector.tensor_tensor(
                            out=pw[:], in0=pw[:], in1=d[:], op=TT.mult
                        )

                # self-edges (d == 0): reference yields inf * 0 = NaN
                mask = wp.tile([P, K], F32)
                nc.vector.tensor_scalar(
                    out=mask[:], in0=d2[:], scalar1=0.0, scalar2=None,
                    op0=TT.is_equal,
                )
                nc.vector.select(
                    out=acc[:], mask=mask[:], on_true=nan_tile[:], on_false=acc[:]
                )

                # ---- radial sin terms ----
                out_tile = iop.tile([P, num_radial * K], F32)
                for kk, fk in enumerate(freqs):
                    fk = _f32(fk)
                    arg = wp.tile([P, K], F32)
                    nc.vector.tensor_scalar(
                        out=arg[:], in0=d[:], scalar1=fk, scalar2=None, op0=TT.mult
                    )
                    # m = round(arg / 2pi) via magic-number trick
                    y = wp.tile([P, K], F32)
                    nc.vector.tensor_scalar(
                        out=y[:], in0=arg[:], scalar1=inv_2pi, scalar2=float(MAGIC_RN),
                        op0=TT.mult, op1=TT.add,
                    )
                    nc.vector.tensor_scalar(
                        out=y[:], in0=y[:], scalar1=float(MAGIC_RN), scalar2=None,
                        op0=TT.subtract,
                    )
                    # r = ((arg - m*c1) - m*c2) - m*c3  in [-pi, pi]
                    red = wp.tile([P, K], F32)
                    nc.vector.cody_waite_cascade(
                        out=red[:], x=arg[:], k=y[:], c1=c1, c2=c2, c3=c3
                    )
                    s = wp.tile([P, K], F32)
                    nc.scalar.activation(
                        out=s[:], in_=red[:], func=mybir.ActivationFunctionType.Sin,
                    )
                    nc.vector.tensor_tensor(
                        out=out_tile[:].rearrange("p (k f) -> p k f", f=num_radial)[
                            :, :, kk
                        ],
                        in0=acc[:],
                        in1=s[:],
                        op=TT.mult,
                    )

                nc.sync.dma_start(out=out_v[rows], in_=out_tile[:])

    nc.compile()
    return nc


def _pick_k(e_core):
    """edges/core = 128 * K * T; prefer K near ~512-1024 with integer T."""
    per_part = e_core // P
    assert e_core % P == 0
    best = None
    for t in range(1, per_part + 1):
        if per_part % t:
            continue
        k = per_part // t
        if k > 2048:
            continue
        score = abs(k - 640)
        if best is None or score < best[0]:
            best = (score, k)
    assert best is not None
    return best[1]


def kernel(R, frequencies, src, dst):
    R = np.ascontiguousarray(np.asarray(R, dtype=np.float32))
    freqs = [float(f) for f in np.asarray(frequencies, dtype=np.float32)]
    src32 = np.ascontiguousarray(np.asarray(src).astype(np.int32))
    dst32 = np.ascontiguousarray(np.asarray(dst).astype(np.int32))

    e_total = src32.shape[0]
    assert e_total % N_CORES == 0
    e_core = e_total // N_CORES
    n_nodes = R.shape[0]
    k_cols = _pick_k(e_core)

    nc = build_module(e_core, n_nodes, freqs, k_cols)

    in_maps = []
    for c in range(N_CORES):
        sl = slice(c * e_core, (c + 1) * e_core)
        in_maps.append({"R": R, "src": src32[sl], "dst": dst32[sl]})

    res = run_bass_kernel_spmd(nc, in_maps, core_ids=list(range(N_CORES)))
    out = np.concatenate([r["out"] for r in res.results], axis=0)
    return out


# revision 6
# speedup vs baseline: 156.6742x; 156.6742x over previous
"""Trainium2 Bass kernel for nn_BesselBasisLayer (DimeNet radial Bessel basis).

Computes, for each edge e:
    diff = R[src[e]] - R[dst[e]]
    dist = sqrt(sum(diff^2))
    d    = dist / CUTOFF
    rbf[e, k] = envelope(d) * sin(freq_k * d)        k = 0..NUM_RADIAL-1
with envelope(x) = 1/x + a*x^5 + b*x^6 + c*x^7 (DimeNet envelope, p = 6).

Strategy: pure edge-parallel sharding across 8 NeuronCores. Each core gets a
contiguous slice of edges (src/dst indices), gathers both endpoint positions
from the full R table in HBM via indirect (descriptor-per-index) DMA, and
computes the RBF with DVE/ACT ops. The R table is passed whole to every core.
"""

import math
import sys

import numpy as np

if "/opt/trn_rl_repo" not in sys.path:
    sys.path.insert(0, "/opt/trn_rl_repo")

import concourse.bacc as bacc
import concourse.bass as bass
import concourse.mybir as mybir
from concourse.bass import IndirectOffsetOnAxis
from concourse.bass_utils import axon_active, run_bass_kernel_spmd
from concourse.tile import TileContext

F32 = mybir.dt.float32
I32 = mybir.dt.int32

N_CORES = 8
CUTOFF = 5.0
ENV_EXPONENT = 5  # DimeNet envelope_exponent; p = exponent + 1 = 6
P = 128  # SBUF partitions

MAGIC_RN = np.float32(1.5 * 2.0**23)  # float32 round-to-nearest-int trick


def _f32(x):
    return float(np.float32(x))


def _cody_waite_2pi():
    """Split 2*pi into three f32 constants with zeroed low mantissa bits so
    that k*c1 and k*c2 are exact for small integer k."""
    two_pi = 2.0 * math.pi

    def chop(x):
        b = np.float32(x).view(np.uint32)
        return (b & np.uint32(0xFFFFF000)).view(np.float32).item()

    c1 = chop(two_pi)
    c2 = chop(two_pi - c1)
    c3 = float(np.float32(two_pi - c1 - c2))
    return c1, c2, c3


def build_module(e_core, n_nodes, freqs, k_cols, num_cores=N_CORES, gather_chunks=5):
    """Build the SPMD Bass module run identically on every core.

    e_core:  edges per core; must equal 128 * k_cols * n_tiles
    n_nodes: rows of R
    freqs:   python floats, the radial frequencies (baked in at trace time)
    k_cols:  edges per partition per tile
    """
    num_radial = len(freqs)
    if k_cols % gather_chunks:
        gather_chunks = 1
    assert e_core % (P * k_cols) == 0, (e_core, k_cols)
    n_tiles = e_core // (P * k_cols)
    K = k_cols

    nc = bacc.Bacc(
        "TRN2",
        target_bir_lowering=False,
        debug=False,
        num_devices=num_cores,
    )

    r_t = nc.dram_tensor("R", [n_nodes, 3], F32, kind="ExternalInput")
    src_t = nc.dram_tensor("src", [e_core], I32, kind="ExternalInput")
    dst_t = nc.dram_tensor("dst", [e_core], I32, kind="ExternalInput")
    out_t = nc.dram_tensor("out", [e_core, num_radial], F32, kind="ExternalOutput")

    r_ap = r_t.ap()[:]
    # view edge arrays as [n_tiles*P, K]; tile t covers rows [t*P, (t+1)*P)
    src_v = src_t.ap()[:].rearrange("(r k) -> r k", k=K)
    dst_v = dst_t.ap()[:].rearrange("(r k) -> r k", k=K)
    # output rows for (tile t, partition p) are contiguous: K*num_radial floats
    out_v = out_t.ap()[:].rearrange("(r k) f -> r (k f)", k=K)

    # envelope coefficients (p = ENV_EXPONENT + 1)
    p_env = ENV_EXPONENT + 1
    coef_a = _f32(-(p_env + 1) * (p_env + 2) / 2.0)
    coef_b = _f32(p_env * (p_env + 2))
    coef_c = _f32(-p_env * (p_env + 1) / 2.0)

    inv_cut2 = _f32(1.0 / (CUTOFF * CUTOFF))
    c1, c2, c3 = _cody_waite_2pi()
    inv_2pi = _f32(1.0 / (2.0 * math.pi))

    TT = mybir.AluOpType
    NAN = float("nan")

    with TileContext(nc) as tc:
        with (
            tc.tile_pool(name="const", bufs=1) as cpool,
            tc.tile_pool(name="io", bufs=2) as iop,
            tc.tile_pool(name="work", bufs=2) as wp,
        ):
            nan_tile = cpool.tile([P, K], F32)
            nc.vector.memset(nan_tile[:], NAN)

            for t in range(n_tiles):
                rows = slice(t * P, (t + 1) * P)

                sidx = iop.tile([P, K], I32)
                didx = iop.tile([P, K], I32)
                nc.sync.dma_start(out=sidx[:], in_=src_v[rows])
                nc.sync.dma_start(out=didx[:], in_=dst_v[rows])

                spos = iop.tile([P, 3 * K], F32)
                dpos = iop.tile([P, 3 * K], F32)
                # chunked so descriptor generation and transfer pipeline
                cw = K // gather_chunks
                assert K % gather_chunks == 0
                for g in range(gather_chunks):
                    cs = slice(g * cw, (g + 1) * cw)
                    ps = slice(3 * g * cw, 3 * (g + 1) * cw)
                    nc.gpsimd.indirect_dma_start(
                        out=spos[:, ps],
                        out_offset=None,
                        in_=r_ap,
                        in_offset=IndirectOffsetOnAxis(ap=sidx[:, cs], axis=0),
                    )
                    nc.gpsimd.indirect_dma_start(
                        out=dpos[:, ps],
                        out_offset=None,
                        in_=r_ap,
                        in_offset=IndirectOffsetOnAxis(ap=didx[:, cs], axis=0),
                    )

                # ---- distance ----
                diff = wp.tile([P, 3 * K], F32)
                nc.vector.tensor_tensor(
                    out=diff[:], in0=spos[:], in1=dpos[:], op=TT.subtract
                )
                nc.vector.tensor_tensor(
                    out=diff[:], in0=diff[:], in1=diff[:], op=TT.mult
                )
                sq3 = diff[:].rearrange("p (k c) -> p k c", c=3)
                d2 = wp.tile([P, K], F32)
                nc.vector.tensor_tensor(
                    out=d2[:], in0=sq3[:, :, 0], in1=sq3[:, :, 1], op=TT.add
                )
                nc.vector.tensor_tensor(
                    out=d2[:], in0=d2[:], in1=sq3[:, :, 2], op=TT.add
                )
                # d = dist / CUTOFF = sqrt(d2 / CUTOFF^2)
                d = wp.tile([P, K], F32)
                nc.scalar.activation(
                    out=d[:], in_=d2[:], func=mybir.ActivationFunctionType.Sqrt,
                    scale=inv_cut2,
                )

                # ---- envelope: 1/d + a*d^5 + b*d^6 + c*d^7 (left-assoc) ----
                acc = wp.tile([P, K], F32)
                nc.vector.reciprocal(out=acc[:], in_=d[:])
                tt = wp.tile([P, K], F32)
                nc.vector.tensor_tensor(out=tt[:], in0=d[:], in1=d[:], op=TT.mult)
                pw = wp.tile([P, K], F32)
                nc.vector.tensor_tensor(out=pw[:], in0=tt[:], in1=tt[:], op=TT.mult)
                nc.vector.tensor_tensor(out=pw[:], in0=pw[:], in1=d[:], op=TT.mult)
                tmp = wp.tile([P, K], F32)
                for coef in (coef_a, coef_b, coef_c):
                    nc.vector.tensor_scalar(
                        out=tmp[:], in0=pw[:], scalar1=coef, scalar2=None, op0=TT.mult
                    )
                    nc.vector.tensor_tensor(
                        out=acc[:], in0=acc[:], in1=tmp[:], op=TT.add
                    )
                    if coef is not coef_c:
                        nc.vector.tensor_tensor(
                            out=pw[:], in0=pw[:], in1=d[:], op=TT.mult
                        )

                # self-edges (d == 0): reference yields inf * 0 = NaN
                mask = wp.tile([P, K], mybir.dt.uint8)
                nc.vector.tensor_scalar(
                    out=mask[:], in0=d2[:], scalar1=0.0, scalar2=None,
                    op0=TT.is_equal,
                )
                nc.vector.select(
                    out=acc[:], mask=mask[:], on_true=nan_tile[:], on_false=acc[:]
                )

                # ---- radial sin terms ----
                out_tile = iop.tile([P, num_radial * K], F32)
                for kk, fk in enumerate(freqs):
                    fk = _f32(fk)
                    arg = wp.tile([P, K], F32)
                    nc.vector.tensor_scalar(
                        out=arg[:], in0=d[:], scalar1=fk, scalar2=None, op0=TT.mult
                    )
                    # m = round(arg / 2pi) via magic-number trick
                    y = wp.tile([P, K], F32)
                    nc.vector.tensor_scalar(
                        out=y[:], in0=arg[:], scalar1=inv_2pi, scalar2=float(MAGIC_RN),
                        op0=TT.mult, op1=TT.add,
                    )
                    nc.vector.tensor_scalar(
                        out=y[:], in0=y[:], scalar1=float(MAGIC_RN), scalar2=None,
                        op0=TT.subtract,
                    )
                    # r = ((arg - m*c1) - m*c2) - m*c3  in [-pi, pi]
                    red = wp.tile([P, K], F32)
                    nc.vector.cody_waite_cascade(
                        out=red[:], x=arg[:], k=y[:], c1=c1, c2=c2, c3=c3
                    )
                    s = wp.tile([P, K], F32)
                    nc.scalar.activation(
                        out=s[:], in_=red[:], func=mybir.ActivationFunctionType.Sin,
                    )
                    nc.vector.tensor_tensor(
                        out=out_tile[:].rearrange("p (k f) -> p k f", f=num_radial)[
                            :, :, kk
                        ],
                        in0=acc[:],
                        in1=s[:],
                        op=TT.mult,
                    )

                nc.sync.dma_start(out=out_v[rows], in_=out_tile[:])

    nc.compile()
    return nc


def _pick_k(e_core):
    """edges/core = 128 * K * T; prefer K near ~512-1024 with integer T."""
    per_part = e_core // P
    assert e_core % P == 0
    best = None
    for t in range(1, per_part + 1):
        if per_part % t:
            continue
        k = per_part // t
        if k > 2048:
            continue
        score = abs(k - 640)
        if best is None or score < best[0]:
            best = (score, k)
    assert best is not None
    return best[1]


# Debug/profiling knobs for the development harness (test.py). The grading
# harness just calls kernel(**inputs) and gets defaults.
TRACE = False
TRACE_KWARGS = {}
LAST_RESULTS = None


def kernel(R, frequencies, src, dst):
    global LAST_RESULTS
    R = np.ascontiguousarray(np.asarray(R, dtype=np.float32))
    freqs = [float(f) for f in np.asarray(frequencies, dtype=np.float32)]
    src32 = np.ascontiguousarray(np.asarray(src).astype(np.int32))
    dst32 = np.ascontiguousarray(np.asarray(dst).astype(np.int32))

    e_total = src32.shape[0]
    assert e_total % N_CORES == 0
    e_core = e_total // N_CORES
    n_nodes = R.shape[0]
    k_cols = _pick_k(e_core)

    nc = build_module(e_core, n_nodes, freqs, k_cols)

    in_maps = []
    for c in range(N_CORES):
        sl = slice(c * e_core, (c + 1) * e_core)
        in_maps.append({"R": R, "src": src32[sl], "dst": dst32[sl]})

    res = run_bass_kernel_spmd(
        nc, in_maps, core_ids=list(range(N_CORES)), trace=TRACE, **TRACE_KWARGS
    )
    LAST_RESULTS = res
    out = np.concatenate([r["out"] for r in res.results], axis=0)
    return out
